# revision 1
# baseline (speedup 1.0000x reference)
"""Single-head self-attention (CrossVit block) on 8 Trainium2 NeuronCores.

Computation (fp32 reference):
    q = x @ Wq + bq ; k = x @ Wk + bk ; v = x @ Wv + bv        [S, E]
    scores = (q @ k^T) / sqrt(E)                               [S, S]
    out = softmax(scores, axis=-1) @ v                         [S, E]
with S = 8192, E = 2048.

Strategy (sequence-parallel over query rows, 1024 per core):
  Host: transpose x -> xT [E, S] (bf16) and hand each core its column
  slice xT_i [E, 1024] plus full Wq/Wk/Wv (bf16) and biases (fp32).
  Device, per core i:
    phase 0: KT_i = Wk^T xT_i (+bk) -> DRAM, AllGather'd in 4 row-slices
             pipelined against the projection itself; V_i = xT_i Wv -> DRAM,
             AllGather'd in 2 slices; QT_i = Wq^T xT_i (+bq) stays in SBUF.
    phase 1: ST[j] = KT_j^T @ QT  (scores transposed: [sk, sq]) -> exp -> SBUF
             l[sq] += ones^T @ exp(ST[j])   (softmax denominators via PE)
    phase 2: O[sq, e] = sum_j exp(ST[j])^T @ V_j  accumulated in PSUM,
             epilogue: O * (1/l) + bv -> out
  Host: concatenate the 8 row blocks.
"""

import math

import numpy as np
import ml_dtypes

S = 8192
E = 2048
N_CORES = 8
SL = S // N_CORES      # 1024 query rows per core
P = 128                # partitions
ECH = E // P           # 16 contraction chunks
JN = S // P            # 64 global key chunks
NQ = 512               # moving free-dim (ISA max per matmul)
QCH = SL // NQ         # 2 query chunks of 512
SCH = SL // P          # 8 query chunks of 128
EQN = E // NQ          # 4 output-column chunks of 512
KAG = 4                # AllGather splits for KT (rows of 512)
VAG = 2                # AllGather splits for V (rows of 512)

_BF16 = ml_dtypes.bfloat16


def _build():
    import concourse.bacc as bacc
    import concourse.bass as bass
    import concourse.tile as tile
    import concourse.mybir as mybir

    bf16 = mybir.dt.bfloat16
    f32 = mybir.dt.float32
    SCALE = 1.0 / math.sqrt(float(E))

    nc = bacc.Bacc("TRN2", target_bir_lowering=False, debug=False,
                   num_devices=N_CORES)

    xt = nc.declare_dram_parameter("xt", [E, SL], bf16, isOutput=False)
    wq = nc.declare_dram_parameter("wq", [ECH, P, ECH, P], bf16, isOutput=False)
    wk = nc.declare_dram_parameter("wk", [ECH, P, ECH, P], bf16, isOutput=False)
    wv = nc.declare_dram_parameter("wv", [E, E], bf16, isOutput=False)
    bq = nc.declare_dram_parameter("bq", [E], f32, isOutput=False)
    bk = nc.declare_dram_parameter("bk", [E], f32, isOutput=False)
    bv = nc.declare_dram_parameter("bv", [E], f32, isOutput=False)
    out = nc.declare_dram_parameter("out", [SL, E], f32, isOutput=True)

    groups = [list(range(N_CORES))]
    KSL = E // KAG        # 512 rows per KT AG slice
    VSL = SL // VAG       # 512 rows per V AG slice

    with tile.TileContext(nc) as tc:
        with (
            tc.tile_pool(name="dram", bufs=1, space="DRAM") as dram,
            tc.tile_pool(name="big", bufs=1) as big,
            tc.tile_pool(name="res", bufs=1) as res,
            tc.tile_pool(name="wstr", bufs=2) as wstr,
            tc.tile_pool(name="kstr", bufs=3) as kstr,
            tc.tile_pool(name="vstr", bufs=3) as vstr,
            tc.tile_pool(name="stg", bufs=2) as stg,
            tc.tile_pool(name="ps", bufs=8, space="PSUM") as ps,
        ):
            kt_in = dram.tile([E, SL], bf16)
            v_in = dram.tile([SL, E], bf16)
            kt_all = [dram.tile([N_CORES * KSL, SL], bf16, addr_space="Shared",
                                name=f"kt_all_{i}") for i in range(KAG)]
            v_all = [dram.tile([N_CORES * VSL, E], bf16, addr_space="Shared",
                               name=f"v_all_{i}") for i in range(VAG)]
            l_dram = dram.tile([1, SL], f32)

            # --- resident SBUF tensors -------------------------------------
            # xt_sb and st_sb share one 128KB/partition slot (disjoint
            # lifetimes: xt only in phase 0, st written in phase 1).
            xt_sb = big.tile([P, ECH, SL], bf16, tag="bigslot")
            qt_sb = res.tile([P, ECH, SL], bf16)
            bq_sb = res.tile([P, ECH], f32)
            bk_sb = res.tile([P, ECH], f32)
            ones_sb = res.tile([P, 1], bf16)

            nc.sync.dma_start(out=xt_sb, in_=xt.rearrange("(c p) s -> p c s", p=P))
            nc.sync.dma_start(out=bq_sb, in_=bq.rearrange("(c p) -> p c", p=P))
            nc.sync.dma_start(out=bk_sb, in_=bk.rearrange("(c p) -> p c", p=P))
            nc.vector.memset(ones_sb, 1.0)

            # --- phase 0a: KT_i = Wk^T @ xT_i + bk -> kt_in, AG in 4 slices -
            def qk_proj(w_param, b_sb, dst_sbuf, dst_dram, wtag, eo_lo, eo_hi):
                for eo in range(eo_lo, eo_hi):
                    w_t = wstr.tile([P, ECH, P], bf16, tag="w",
                                    name=f"w_{wtag}_{eo}")
                    nc.sync.dma_start(out=w_t, in_=w_param[eo])
                    for q in range(QCH):
                        acc = ps.tile([P, NQ], f32, tag="mm",
                                      name=f"acc_{wtag}_{eo}_{q}")
                        for ec in range(ECH):
                            nc.tensor.matmul(
                                acc, w_t[:, ec],
                                xt_sb[:, ec, q * NQ:(q + 1) * NQ],
                                start=(ec == 0), stop=(ec == ECH - 1))
                        if dst_sbuf is not None:
                            nc.scalar.activation(
                                dst_sbuf[:, eo, q * NQ:(q + 1) * NQ], acc,
                                mybir.ActivationFunctionType.Identity,
                                bias=b_sb[:, eo:eo + 1], scale=1.0)
                        else:
                            kstg = stg.tile([P, NQ], bf16, tag="kstg",
                                            name=f"kstg_{eo}_{q}")
                            nc.scalar.activation(
                                kstg, acc,
                                mybir.ActivationFunctionType.Identity,
                                bias=b_sb[:, eo:eo + 1], scale=1.0)
                            nc.sync.dma_start(
                                out=dst_dram[eo * P:(eo + 1) * P,
                                             q * NQ:(q + 1) * NQ],
                                in_=kstg)

            eo_per_slice = KSL // P  # 4
            for i in range(KAG):
                qk_proj(wk, bk_sb, None, kt_in, "wk",
                        i * eo_per_slice, (i + 1) * eo_per_slice)
                nc.gpsimd.collective_compute(
                    "AllGather", mybir.AluOpType.bypass, replica_groups=groups,
                    ins=[kt_in[i * KSL:(i + 1) * KSL, :].opt()],
                    outs=[kt_all[i].opt()])

            # --- phase 0b: V_i = xT_i^T @ Wv -> v_in, AG in 2 slices --------
            # bv is folded into the epilogue (attn rows sum to 1).
            for h in range(VAG):            # s-halves (4 s-chunks each)
                for eq in range(EQN):       # e-quarters of 512
                    accs = [ps.tile([P, NQ], f32, tag="mm",
                                    name=f"vacc_{h}_{eq}_{si}")
                            for si in range(4)]
                    for ec in range(ECH):
                        wv_t = vstr.tile([P, NQ], bf16, tag="wv",
                                         name=f"wv_{h}_{eq}_{ec}")
                        nc.sync.dma_start(
                            out=wv_t,
                            in_=wv[ec * P:(ec + 1) * P,
                                   eq * NQ:(eq + 1) * NQ])
                        for si in range(4):
                            s = h * 4 + si
                            nc.tensor.matmul(
                                accs[si], xt_sb[:, ec, s * P:(s + 1) * P],
                                wv_t, start=(ec == 0), stop=(ec == ECH - 1))
                    for si in range(4):
                        s = h * 4 + si
                        vstg = stg.tile([P, NQ], bf16, tag="vstg",
                                        name=f"vstg_{h}_{eq}_{si}")
                        nc.vector.tensor_copy(out=vstg, in_=accs[si])
                        nc.sync.dma_start(
                            out=v_in[s * P:(s + 1) * P, eq * NQ:(eq + 1) * NQ],
                            in_=vstg)
                nc.gpsimd.collective_compute(
                    "AllGather", mybir.AluOpType.bypass, replica_groups=groups,
                    ins=[v_in[h * VSL:(h + 1) * VSL, :].opt()],
                    outs=[v_all[h].opt()])

            # --- phase 0c: QT_i = Wq^T @ xT_i + bq -> qt_sb (SBUF-resident) -
            qk_proj(wq, bq_sb, qt_sb, None, "wq", 0, ECH)

            # --- phase 1: ST[j] = KT_j^T @ QT, exp, l accumulation ----------
            st_sb = big.tile([P, JN, SL], bf16, tag="bigslot")
            l_ps = [ps.tile([1, NQ], f32, tag="mm", name=f"l_{q}")
                    for q in range(QCH)]
            ec_per_slice = ECH // KAG  # 4 e-chunks per AG slice
            for j in range(JN):
                r, c = j // SCH, j % SCH
                kt_t = kstr.tile([P, ECH, P], bf16, tag="kt", name=f"kt_{j}")
                for i in range(KAG):
                    nc.sync.dma_start(
                        out=kt_t[:, i * ec_per_slice:(i + 1) * ec_per_slice, :],
                        in_=kt_all[i][r * KSL:(r + 1) * KSL,
                                      c * P:(c + 1) * P].rearrange(
                                          "(ec p) s -> p ec s", p=P))
                for q in range(QCH):
                    st_ps = ps.tile([P, NQ], f32, tag="mm", name=f"st_{j}_{q}")
                    for ec in range(ECH):
                        nc.tensor.matmul(
                            st_ps, kt_t[:, ec],
                            qt_sb[:, ec, q * NQ:(q + 1) * NQ],
                            start=(ec == 0), stop=(ec == ECH - 1))
                    nc.scalar.activation(
                        st_sb[:, j, q * NQ:(q + 1) * NQ], st_ps,
                        mybir.ActivationFunctionType.Exp, scale=SCALE)
                    nc.tensor.matmul(
                        l_ps[q], ones_sb, st_sb[:, j, q * NQ:(q + 1) * NQ],
                        start=(j == 0), stop=(j == JN - 1))

            # --- between phases: l -> reciprocal, per-partition layout ------
            l_row = res.tile([1, SL], f32)
            for q in range(QCH):
                nc.vector.tensor_copy(out=l_row[:, q * NQ:(q + 1) * NQ],
                                      in_=l_ps[q])
            nc.sync.dma_start(out=l_dram, in_=l_row)
            l_pp = res.tile([P, SCH], f32)
            nc.sync.dma_start(out=l_pp,
                              in_=l_dram[0].rearrange("(c p) -> p c", p=P))
            recip = res.tile([P, SCH], f32)
            nc.vector.reciprocal(recip, l_pp)
            bv_sb = res.tile([P, NQ], f32)
            _bv_ap = bv.ap()

            # --- phase 2: O = exp(ST)^T @ V, epilogue -----------------------
            for eq in range(EQN):
                bv_bcast_ap = bass.AP(tensor=_bv_ap.tensor,
                                      offset=_bv_ap.offset + eq * NQ,
                                      ap=[[0, P], [1, NQ]])
                nc.sync.dma_start(out=bv_sb, in_=bv_bcast_ap)
                o_ps = [ps.tile([P, NQ], f32, tag="mm", name=f"o_{eq}_{s}")
                        for s in range(SCH)]
                for j in range(JN):
                    r, sloc = j // SCH, (j % SCH) * P
                    h, off = sloc // VSL, sloc % VSL
                    v_t = vstr.tile([P, NQ], bf16, tag="v", name=f"v_{eq}_{j}")
                    nc.sync.dma_start(
                        out=v_t,
                        in_=v_all[h][r * VSL + off:r * VSL + off + P,
                                     eq * NQ:(eq + 1) * NQ])
                    for s in range(SCH):
                        nc.tensor.matmul(
                            o_ps[s], st_sb[:, j, s * P:(s + 1) * P], v_t,
                            start=(j == 0), stop=(j == JN - 1))
                for s in range(SCH):
                    o_stg = stg.tile([P, NQ], f32, tag="ostg",
                                     name=f"ostg_{eq}_{s}")
                    nc.vector.tensor_scalar_mul(o_stg, o_ps[s],
                                                recip[:, s:s + 1])
                    nc.vector.tensor_tensor(
                        out=o_stg, in0=o_stg, in1=bv_sb,
                        op=mybir.AluOpType.add)
                    nc.sync.dma_start(
                        out=out[s * P:(s + 1) * P, eq * NQ:(eq + 1) * NQ],
                        in_=o_stg)

    nc.compile()
    return nc


def kernel(x, Wq, bq, Wk, bk, Wv, bv):
    from concourse.bass_utils import run_bass_kernel_spmd

    xt = np.ascontiguousarray(x.astype(_BF16).T)          # [E, S] bf16

    def _pre(w):  # [e_in, e_out] -> [eo, p, c, n] so each eo-slice is contiguous
        return np.ascontiguousarray(
            w.astype(_BF16).reshape(ECH, P, ECH, P).transpose(2, 1, 0, 3))

    wqb = _pre(Wq)
    wkb = _pre(Wk)
    wvb = np.ascontiguousarray(Wv.astype(_BF16))
    bqf = np.ascontiguousarray(bq.astype(np.float32))
    bkf = np.ascontiguousarray(bk.astype(np.float32))
    bvf = np.ascontiguousarray(bv.astype(np.float32))

    in_maps = []
    for r in range(N_CORES):
        in_maps.append({
            "xt": np.ascontiguousarray(xt[:, r * SL:(r + 1) * SL]),
            "wq": wqb, "wk": wkb, "wv": wvb,
            "bq": bqf, "bk": bkf, "bv": bvf,
        })

    nc = _build()
    res = run_bass_kernel_spmd(nc, in_maps, core_ids=list(range(N_CORES)))
    global LAST_RESULT
    LAST_RESULT = res
    return np.concatenate([res.results[r]["out"] for r in range(N_CORES)],
                          axis=0).astype(np.float32)


LAST_RESULT = None



# revision 3
# speedup vs baseline: 1.1598x; 1.1598x over previous
"""Single-head self-attention (CrossVit block) on 8 Trainium2 NeuronCores.

Computation (fp32 reference):
    q = x @ Wq + bq ; k = x @ Wk + bk ; v = x @ Wv + bv        [S, E]
    scores = (q @ k^T) / sqrt(E)                               [S, S]
    out = softmax(scores, axis=-1) @ v                         [S, E]
with S = 8192, E = 2048.

Key algebraic rewrite (host folds weights, device never computes K):
    q_i . k_j = x_i (Wq Wk^T) x_j^T + x_i.(Wq bk) + x_j.(Wk bq) + bq.bk
The x_i.(Wq bk) and bq.bk terms are constant per query row -> cancel in
softmax.  So with M = Wq Wk^T (host-precomputed, weight-only) and
c_j = SCALE * x_j.(Wk bq):
    softmax-arg_ij = SCALE * (x M x^T)_ij + c_j
This removes the K projection AND the K AllGather entirely; phase 1
contracts against the raw x^T input streamed from DRAM.

Per core i (1024 query rows):
  phase 0: YT_i = M^T xT_i  (= (x_i M)^T) -> SBUF-resident
           c_i  = x_i . (SCALE Wk bq) -> tiny AllGather -> c_pp [128,64]
           V_i  = x_i Wv -> DRAM, AllGather'd in 2 row-halves
  phase 1: ST[j] = xT_j^T @ YT   ([sk,sq]), exp(. + c) -> SBUF (bf16)
           l[sq] += ones^T @ exp(ST[j])  (deferred one j to avoid stalls)
  phase 2: O[sq,e] = sum_j exp(ST[j])^T @ V_j  in PSUM,
           epilogue: O * (1/l) + bv -> out
Host: concatenate the 8 row blocks.
"""

import math

import numpy as np
import ml_dtypes

S = 8192
E = 2048
N_CORES = 8
SL = S // N_CORES      # 1024 query rows per core
P = 128                # partitions
ECH = E // P           # 16 contraction chunks
JN = S // P            # 64 global key chunks
NQ = 512               # moving free-dim (ISA max per matmul)
QCH = SL // NQ         # 2 query chunks of 512
SCH = SL // P          # 8 query chunks of 128
EQN = E // NQ          # 4 output-column chunks of 512
VAG = 2                # AllGather splits for V (row-halves of 512)

_BF16 = ml_dtypes.bfloat16
SCALE = 1.0 / math.sqrt(float(E))


def _build():
    import concourse.bacc as bacc
    import concourse.bass as bass
    import concourse.tile as tile
    import concourse.mybir as mybir

    bf16 = mybir.dt.bfloat16
    f32 = mybir.dt.float32

    nc = bacc.Bacc("TRN2", target_bir_lowering=False, debug=False,
                   num_devices=N_CORES)

    xt = nc.declare_dram_parameter("xt", [E, SL], bf16, isOutput=False)
    x4d = nc.declare_dram_parameter("x4d", [JN, P, ECH, P], bf16,
                                    isOutput=False)
    wm = nc.declare_dram_parameter("wm", [ECH, P, ECH, P], bf16,
                                   isOutput=False)
    wv = nc.declare_dram_parameter("wv", [E, E], bf16, isOutput=False)
    wc = nc.declare_dram_parameter("wc", [E], bf16, isOutput=False)
    bv = nc.declare_dram_parameter("bv", [E], f32, isOutput=False)
    out = nc.declare_dram_parameter("out", [SL, E], f32, isOutput=True)

    groups = [list(range(N_CORES))]
    VSL = SL // VAG       # 512 rows per V AG slice

    with tile.TileContext(nc) as tc:
        with (
            tc.tile_pool(name="dram", bufs=1, space="DRAM") as dram,
            tc.tile_pool(name="big", bufs=1) as big,
            tc.tile_pool(name="res", bufs=1) as res,
            tc.tile_pool(name="wstr", bufs=2) as wstr,
            tc.tile_pool(name="kstr", bufs=3) as kstr,
            tc.tile_pool(name="vstr", bufs=3) as vstr,
            tc.tile_pool(name="stg", bufs=2) as stg,
            tc.tile_pool(name="ps", bufs=8, space="PSUM") as ps,
        ):
            v_in = dram.tile([SL, E], bf16)
            v_all = [dram.tile([N_CORES * VSL, E], bf16, addr_space="Shared",
                               name=f"v_all_{i}") for i in range(VAG)]
            c_loc = dram.tile([1, SL], f32)
            c_gath = dram.tile([N_CORES, SL], f32, addr_space="Shared")
            l_dram = dram.tile([1, SL], f32)

            # --- resident SBUF tensors -------------------------------------
            # xt_sb and st_sb share one 128KB/partition slot (disjoint
            # lifetimes: xt only in phase 0, st written in phase 1).
            xt_sb = big.tile([P, ECH, SL], bf16, tag="bigslot")
            qt_sb = res.tile([P, ECH, SL], bf16)
            wc_sb = res.tile([P, ECH], bf16)
            ones_sb = res.tile([P, 1], bf16)
            c_pp = res.tile([P, JN], f32)

            nc.sync.dma_start(out=wc_sb, in_=wc.rearrange("(c p) -> p c", p=P))
            for ec in range(ECH):
                nc.sync.dma_start(out=xt_sb[:, ec],
                                  in_=xt[ec * P:(ec + 1) * P, :])
            nc.vector.memset(ones_sb, 1.0)

            # --- phase 0a: YT_i = M^T @ xT_i -> qt_sb (SBUF-resident) -------
            for eo in range(ECH):
                w_t = wstr.tile([P, ECH, P], bf16, tag="w", name=f"w_{eo}")
                nc.sync.dma_start(out=w_t, in_=wm[eo])
                accs = [ps.tile([P, NQ], f32, tag="mm", name=f"qacc_{eo}_{q}")
                        for q in range(QCH)]
                for ec in range(ECH):
                    for q in range(QCH):
                        nc.tensor.matmul(
                            accs[q], w_t[:, ec],
                            xt_sb[:, ec, q * NQ:(q + 1) * NQ],
                            start=(ec == 0), stop=(ec == ECH - 1))
                for q in range(QCH):
                    nc.vector.tensor_copy(
                        out=qt_sb[:, eo, q * NQ:(q + 1) * NQ], in_=accs[q])

            # --- phase 0b: c_i = x_i . wc -> AllGather (tiny) ---------------
            c_ps = [ps.tile([1, NQ], f32, tag="mm", name=f"c_{q}")
                    for q in range(QCH)]
            for q in range(QCH):
                for ec in range(ECH):
                    nc.tensor.matmul(
                        c_ps[q], wc_sb[:, ec:ec + 1],
                        xt_sb[:, ec, q * NQ:(q + 1) * NQ],
                        start=(ec == 0), stop=(ec == ECH - 1))
            c_row = res.tile([1, SL], f32)
            for q in range(QCH):
                nc.vector.tensor_copy(out=c_row[:, q * NQ:(q + 1) * NQ],
                                      in_=c_ps[q])
            nc.sync.dma_start(out=c_loc, in_=c_row)
            nc.gpsimd.collective_compute(
                "AllGather", mybir.AluOpType.bypass, replica_groups=groups,
                ins=[c_loc.opt()], outs=[c_gath.opt()])
            nc.sync.dma_start(
                out=c_pp,
                in_=c_gath.rearrange("r (j p) -> p (r j)", p=P))

            # --- phase 0c: V_i = x_i @ Wv -> v_in, AG in 2 row-halves -------
            # bv is folded into the epilogue (attn rows sum to 1).
            for h in range(VAG):            # s-halves (4 s-chunks each)
                for eq in range(EQN):       # e-quarters of 512
                    accs = [ps.tile([P, NQ], f32, tag="mm",
                                    name=f"vacc_{h}_{eq}_{si}")
                            for si in range(4)]
                    for ec in range(ECH):
                        wv_t = vstr.tile([P, NQ], bf16, tag="wv",
                                         name=f"wv_{h}_{eq}_{ec}")
                        nc.sync.dma_start(
                            out=wv_t,
                            in_=wv[ec * P:(ec + 1) * P,
                                   eq * NQ:(eq + 1) * NQ])
                        for si in range(4):
                            s = h * 4 + si
                            nc.tensor.matmul(
                                accs[si], xt_sb[:, ec, s * P:(s + 1) * P],
                                wv_t, start=(ec == 0), stop=(ec == ECH - 1))
                    for si in range(4):
                        s = h * 4 + si
                        vstg = stg.tile([P, NQ], bf16, tag="vstg",
                                        name=f"vstg_{h}_{eq}_{si}")
                        nc.vector.tensor_copy(out=vstg, in_=accs[si])
                        nc.sync.dma_start(
                            out=v_in[s * P:(s + 1) * P, eq * NQ:(eq + 1) * NQ],
                            in_=vstg)
                nc.gpsimd.collective_compute(
                    "AllGather", mybir.AluOpType.bypass, replica_groups=groups,
                    ins=[v_in[h * VSL:(h + 1) * VSL, :].opt()],
                    outs=[v_all[h].opt()])

            # --- phase 1: ST[j] = xT_j^T @ YT, exp(.+c), l accumulation -----
            # l-matmuls are deferred one j so they never wait on the Scalar
            # engine's exp output.
            st_sb = big.tile([P, JN, SL], bf16, tag="bigslot")
            l_ps = [ps.tile([1, NQ], f32, tag="mm", name=f"l_{q}")
                    for q in range(QCH)]

            def l_mm(j):
                for q in range(QCH):
                    nc.tensor.matmul(
                        l_ps[q], ones_sb, st_sb[:, j, q * NQ:(q + 1) * NQ],
                        start=(j == 0), stop=(j == JN - 1))

            for j in range(JN):
                kt_t = kstr.tile([P, ECH, P], bf16, tag="kt", name=f"kt_{j}")
                nc.sync.dma_start(out=kt_t, in_=x4d[j])
                for q in range(QCH):
                    st_ps = ps.tile([P, NQ], f32, tag="mm", name=f"st_{j}_{q}")
                    for ec in range(ECH):
                        nc.tensor.matmul(
                            st_ps, kt_t[:, ec],
                            qt_sb[:, ec, q * NQ:(q + 1) * NQ],
                            start=(ec == 0), stop=(ec == ECH - 1))
                    nc.scalar.activation(
                        st_sb[:, j, q * NQ:(q + 1) * NQ], st_ps,
                        mybir.ActivationFunctionType.Exp,
                        bias=c_pp[:, j:j + 1], scale=SCALE)
                if j > 0:
                    l_mm(j - 1)
            l_mm(JN - 1)

            # --- between phases: l -> reciprocal, per-partition layout ------
            l_row = res.tile([1, SL], f32)
            for q in range(QCH):
                nc.vector.tensor_copy(out=l_row[:, q * NQ:(q + 1) * NQ],
                                      in_=l_ps[q])
            nc.sync.dma_start(out=l_dram, in_=l_row)
            l_pp = res.tile([P, SCH], f32)
            nc.sync.dma_start(out=l_pp,
                              in_=l_dram[0].rearrange("(c p) -> p c", p=P))
            recip = res.tile([P, SCH], f32)
            nc.vector.reciprocal(recip, l_pp)
            bv_sb = res.tile([P, NQ], f32)
            _bv_ap = bv.ap()

            # --- phase 2: O = exp(ST)^T @ V, epilogue -----------------------
            for eq in range(EQN):
                bv_bcast_ap = bass.AP(tensor=_bv_ap.tensor,
                                      offset=_bv_ap.offset + eq * NQ,
                                      ap=[[0, P], [1, NQ]])
                nc.sync.dma_start(out=bv_sb, in_=bv_bcast_ap)
                o_ps = [ps.tile([P, NQ], f32, tag="mm", name=f"o_{eq}_{s}")
                        for s in range(SCH)]
                for j in range(JN):
                    r, sloc = j // SCH, (j % SCH) * P
                    h, off = sloc // VSL, sloc % VSL
                    v_t = vstr.tile([P, NQ], bf16, tag="v", name=f"v_{eq}_{j}")
                    nc.sync.dma_start(
                        out=v_t,
                        in_=v_all[h][r * VSL + off:r * VSL + off + P,
                                     eq * NQ:(eq + 1) * NQ])
                    for s in range(SCH):
                        nc.tensor.matmul(
                            o_ps[s], st_sb[:, j, s * P:(s + 1) * P], v_t,
                            start=(j == 0), stop=(j == JN - 1))
                for s in range(SCH):
                    o_stg = stg.tile([P, NQ], f32, tag="ostg",
                                     name=f"ostg_{eq}_{s}")
                    nc.vector.tensor_scalar_mul(o_stg, o_ps[s],
                                                recip[:, s:s + 1])
                    nc.vector.tensor_tensor(
                        out=o_stg, in0=o_stg, in1=bv_sb,
                        op=mybir.AluOpType.add)
                    nc.sync.dma_start(
                        out=out[s * P:(s + 1) * P, eq * NQ:(eq + 1) * NQ],
                        in_=o_stg)

    nc.compile()
    return nc


def kernel(x, Wq, bq, Wk, bk, Wv, bv):
    from concourse.bass_utils import run_bass_kernel_spmd

    M = (np.asarray(Wq, dtype=np.float64)
         @ np.asarray(Wk, dtype=np.float64).T)           # [E, E] weight-only
    wc = SCALE * (np.asarray(Wk, dtype=np.float64)
                  @ np.asarray(bq, dtype=np.float64))    # [E] weight-only

    xb = x.astype(_BF16)                                  # [S, E]
    xt = np.ascontiguousarray(xb.T)                       # [E, S] bf16
    # x4d[j, p, c, s] = xT[c*128+p, j*128+s]: contiguous 512KB per j-tile
    x4d = np.ascontiguousarray(
        xt.reshape(ECH, P, JN, P).transpose(2, 1, 0, 3))

    def _pre(w):  # [e_in, e_out] -> [eo, p, c, n] so each eo-slice is contiguous
        return np.ascontiguousarray(
            w.astype(_BF16).reshape(ECH, P, ECH, P).transpose(2, 1, 0, 3))

    wmb = _pre(M.astype(np.float32))
    wvb = np.ascontiguousarray(Wv.astype(_BF16))
    wcb = np.ascontiguousarray(wc.astype(_BF16))
    bvf = np.ascontiguousarray(bv.astype(np.float32))

    in_maps = []
    for r in range(N_CORES):
        in_maps.append({
            "xt": np.ascontiguousarray(xt[:, r * SL:(r + 1) * SL]),
            "x4d": x4d,
            "wm": wmb, "wv": wvb, "wc": wcb, "bv": bvf,
        })

    nc = _build()
    res = run_bass_kernel_spmd(nc, in_maps, core_ids=list(range(N_CORES)))
    global LAST_RESULT
    LAST_RESULT = res
    return np.concatenate([res.results[r]["out"] for r in range(N_CORES)],
                          axis=0).astype(np.float32)


LAST_RESULT = None


# revision 14
# speedup vs baseline: 1.1653x; 1.0048x over previous
"""Single-head self-attention (CrossVit block) on 8 Trainium2 NeuronCores.

Computation (fp32 reference):
    q = x @ Wq + bq ; k = x @ Wk + bk ; v = x @ Wv + bv        [S, E]
    scores = (q @ k^T) / sqrt(E)                               [S, S]
    out = softmax(scores, axis=-1) @ v                         [S, E]
with S = 8192, E = 2048.

Key algebraic rewrite (host folds weights, device never computes K):
    q_i . k_j = x_i (Wq Wk^T) x_j^T + x_i.(Wq bk) + x_j.(Wk bq) + bq.bk
The x_i.(Wq bk) and bq.bk terms are constant per query row -> cancel in
softmax.  So with M = Wq Wk^T (host-precomputed, weight-only) and
c_j = SCALE * x_j.(Wk bq):
    softmax-arg_ij = SCALE * (x M x^T)_ij + c_j
This removes the K projection AND the K AllGather entirely; phase 1
contracts against the raw x^T input streamed from DRAM.

Per core i (1024 query rows):
  phase 0: YT_i = M^T xT_i  (= (x_i M)^T) -> SBUF-resident
           c_i  = x_i . (SCALE Wk bq) -> tiny AllGather -> c_pp [128,64]
           V_i  = x_i Wv -> DRAM, AllGather'd in 2 row-halves
  phase 1: ST[j] = xT_j^T @ YT   ([sk,sq]), exp(. + c) -> SBUF (bf16)
           l[sq] += ones^T @ exp(ST[j])  (deferred one j to avoid stalls)
  phase 2: O[sq,e] = sum_j exp(ST[j])^T @ V_j  in PSUM,
           epilogue: O * (1/l) + bv -> out
Host: concatenate the 8 row blocks.
"""

import math

import numpy as np
import ml_dtypes

S = 8192
E = 2048
N_CORES = 8
SL = S // N_CORES      # 1024 query rows per core
P = 128                # partitions
ECH = E // P           # 16 contraction chunks
JN = S // P            # 64 global key chunks
NQ = 512               # moving free-dim (ISA max per matmul)
QCH = SL // NQ         # 2 query chunks of 512
SCH = SL // P          # 8 query chunks of 128
EQN = E // NQ          # 4 output-column chunks of 512
VAG = 2                # AllGather splits for V (row-halves of 512)

_BF16 = ml_dtypes.bfloat16
SCALE = 1.0 / math.sqrt(float(E))


def _build():
    import concourse.bacc as bacc
    import concourse.bass as bass
    import concourse.tile as tile
    import concourse.mybir as mybir

    bf16 = mybir.dt.bfloat16
    f32 = mybir.dt.float32

    nc = bacc.Bacc("TRN2", target_bir_lowering=False, debug=False,
                   num_devices=N_CORES)

    xt = nc.declare_dram_parameter("xt", [E, SL], bf16, isOutput=False)
    x4d = nc.declare_dram_parameter("x4d", [JN, P, ECH, P], bf16,
                                    isOutput=False)
    wm = nc.declare_dram_parameter("wm", [ECH, P, ECH, P], bf16,
                                   isOutput=False)
    wv = nc.declare_dram_parameter("wv", [E, E], bf16, isOutput=False)
    wc = nc.declare_dram_parameter("wc", [E], bf16, isOutput=False)
    bv = nc.declare_dram_parameter("bv", [E], f32, isOutput=False)
    out = nc.declare_dram_parameter("out", [SL, E], f32, isOutput=True)

    groups = [list(range(N_CORES))]
    VSL = SL // VAG       # 512 rows per V AG slice

    with tile.TileContext(nc) as tc:
        with (
            tc.tile_pool(name="dram", bufs=1, space="DRAM") as dram,
            tc.tile_pool(name="big", bufs=1) as big,
            tc.tile_pool(name="res", bufs=1) as res,
            tc.tile_pool(name="wstr", bufs=2) as wstr,
            tc.tile_pool(name="kstr", bufs=5) as kstr,
            tc.tile_pool(name="vstr", bufs=3) as vstr,
            tc.tile_pool(name="stg", bufs=2) as stg,
            tc.tile_pool(name="ps", bufs=8, space="PSUM") as ps,
        ):
            v_in = dram.tile([SL, E], bf16)
            v_all = [dram.tile([N_CORES * VSL, E], bf16, addr_space="Shared",
                               name=f"v_all_{i}") for i in range(VAG)]
            c_loc = dram.tile([1, SL], f32)
            c_gath = dram.tile([N_CORES, SL], f32, addr_space="Shared")
            l_dram = dram.tile([1, SL], f32)

            # --- resident SBUF tensors -------------------------------------
            # xt_sb and st_sb share one 128KB/partition slot (disjoint
            # lifetimes: xt only in phase 0, st written in phase 1).
            xt_sb = big.tile([P, ECH, SL], bf16, tag="bigslot")
            qt_sb = res.tile([P, ECH, SL], bf16)
            wc_sb = res.tile([P, ECH], bf16)
            ones_sb = res.tile([P, 1], bf16)
            c_pp = res.tile([P, JN], f32)

            # First weight tile ahead of the bulk x load so matmuls can
            # start immediately; x chunks split across both HWDGE queues.
            w_t0 = wstr.tile([P, ECH, P], bf16, tag="w", name="w_0")
            nc.sync.dma_start(out=w_t0, in_=wm[0])
            nc.sync.dma_start(out=wc_sb, in_=wc.rearrange("(c p) -> p c", p=P))
            for ec in range(ECH):
                eng = nc.sync if ec % 2 == 0 else nc.scalar
                eng.dma_start(out=xt_sb[:, ec],
                              in_=xt[ec * P:(ec + 1) * P, :])
            nc.vector.memset(ones_sb, 1.0)

            # --- phase 0a: YT_i = M^T @ xT_i -> qt_sb (SBUF-resident) -------
            for eo in range(ECH):
                if eo == 0:
                    w_t = w_t0
                else:
                    w_t = wstr.tile([P, ECH, P], bf16, tag="w",
                                    name=f"w_{eo}")
                    nc.sync.dma_start(out=w_t, in_=wm[eo])
                accs = [ps.tile([P, NQ], f32, tag="mm", name=f"qacc_{eo}_{q}")
                        for q in range(QCH)]
                for ec in range(ECH):
                    for q in range(QCH):
                        nc.tensor.matmul(
                            accs[q], w_t[:, ec],
                            xt_sb[:, ec, q * NQ:(q + 1) * NQ],
                            start=(ec == 0), stop=(ec == ECH - 1))
                for q in range(QCH):
                    nc.vector.tensor_copy(
                        out=qt_sb[:, eo, q * NQ:(q + 1) * NQ], in_=accs[q])

            # --- phase 0b: c_i = x_i . wc -> AllGather (tiny) ---------------
            c_ps = [ps.tile([1, NQ], f32, tag="mm", name=f"c_{q}")
                    for q in range(QCH)]
            for q in range(QCH):
                for ec in range(ECH):
                    nc.tensor.matmul(
                        c_ps[q], wc_sb[:, ec:ec + 1],
                        xt_sb[:, ec, q * NQ:(q + 1) * NQ],
                        start=(ec == 0), stop=(ec == ECH - 1))
            c_row = res.tile([1, SL], f32, tag="row")
            for q in range(QCH):
                nc.vector.tensor_copy(out=c_row[:, q * NQ:(q + 1) * NQ],
                                      in_=c_ps[q])
            nc.sync.dma_start(out=c_loc, in_=c_row)
            nc.gpsimd.collective_compute(
                "AllGather", mybir.AluOpType.bypass, replica_groups=groups,
                ins=[c_loc.opt()], outs=[c_gath.opt()])
            nc.sync.dma_start(
                out=c_pp,
                in_=c_gath.rearrange("r (j p) -> p (r j)", p=P))

            # --- phase 0c: V_i = x_i @ Wv -> v_in, AG in 2 row-halves -------
            # bv is folded into the epilogue (attn rows sum to 1).
            for h in range(VAG):            # s-halves (4 s-chunks each)
                for eq in range(EQN):       # e-quarters of 512
                    accs = [ps.tile([P, NQ], f32, tag="mm",
                                    name=f"vacc_{h}_{eq}_{si}")
                            for si in range(4)]
                    for ec in range(ECH):
                        wv_t = vstr.tile([P, NQ], bf16, tag="vv",
                                         name=f"wv_{h}_{eq}_{ec}")
                        nc.sync.dma_start(
                            out=wv_t,
                            in_=wv[ec * P:(ec + 1) * P,
                                   eq * NQ:(eq + 1) * NQ])
                        for si in range(4):
                            s = h * 4 + si
                            nc.tensor.matmul(
                                accs[si], xt_sb[:, ec, s * P:(s + 1) * P],
                                wv_t, start=(ec == 0), stop=(ec == ECH - 1))
                    for si in range(4):
                        s = h * 4 + si
                        vstg = stg.tile([P, NQ], bf16, tag="vstg",
                                        name=f"vstg_{h}_{eq}_{si}")
                        nc.vector.tensor_copy(out=vstg, in_=accs[si])
                        nc.sync.dma_start(
                            out=v_in[s * P:(s + 1) * P, eq * NQ:(eq + 1) * NQ],
                            in_=vstg)
                nc.gpsimd.collective_compute(
                    "AllGather", mybir.AluOpType.bypass, replica_groups=groups,
                    ins=[v_in[h * VSL:(h + 1) * VSL, :].opt()],
                    outs=[v_all[h].opt()])

            # --- phase 1: ST[j] = xT_j^T @ YT, exp(.+c), l accumulation -----
            # l-matmuls are deferred one j so they never wait on the Scalar
            # engine's exp output.
            st_sb = big.tile([P, JN, SL], bf16, tag="bigslot")
            l_ps = [ps.tile([1, NQ], f32, tag="mm", name=f"l_{q}")
                    for q in range(QCH)]

            def l_mm(j):
                for q in range(QCH):
                    nc.tensor.matmul(
                        l_ps[q], ones_sb, st_sb[:, j, q * NQ:(q + 1) * NQ],
                        start=(j == 0), stop=(j == JN - 1))

            for j in range(JN):
                kt_t = kstr.tile([P, ECH, P], bf16, tag="kt", name=f"kt_{j}")
                nc.scalar.dma_start(out=kt_t, in_=x4d[j])
                for q in range(QCH):
                    st_ps = ps.tile([P, NQ], f32, tag="mm", name=f"st_{j}_{q}")
                    for ec in range(ECH):
                        nc.tensor.matmul(
                            st_ps, kt_t[:, ec],
                            qt_sb[:, ec, q * NQ:(q + 1) * NQ],
                            start=(ec == 0), stop=(ec == ECH - 1))
                    nc.scalar.activation(
                        st_sb[:, j, q * NQ:(q + 1) * NQ], st_ps,
                        mybir.ActivationFunctionType.Exp,
                        bias=c_pp[:, j:j + 1], scale=SCALE)
                if j > 0:
                    l_mm(j - 1)
            l_mm(JN - 1)

            # --- between phases: l -> reciprocal, per-partition layout ------
            l_row = res.tile([1, SL], f32, tag="row")
            for q in range(QCH):
                nc.vector.tensor_copy(out=l_row[:, q * NQ:(q + 1) * NQ],
                                      in_=l_ps[q])
            nc.sync.dma_start(out=l_dram, in_=l_row)
            l_pp = res.tile([P, SCH], f32)
            nc.sync.dma_start(out=l_pp,
                              in_=l_dram[0].rearrange("(c p) -> p c", p=P))
            recip = res.tile([P, SCH], f32)
            nc.vector.reciprocal(recip, l_pp)
            _bv_ap = bv.ap()

            # --- phase 2: O = exp(ST)^T @ V, epilogue -----------------------
            for eq in range(EQN):
                bv_sb = stg.tile([P, NQ], f32, tag="bv", name=f"bv_{eq}")
                bv_bcast_ap = bass.AP(tensor=_bv_ap.tensor,
                                      offset=_bv_ap.offset + eq * NQ,
                                      ap=[[0, P], [1, NQ]])
                nc.sync.dma_start(out=bv_sb, in_=bv_bcast_ap)
                o_ps = [ps.tile([P, NQ], f32, tag="mm", name=f"o_{eq}_{s}")
                        for s in range(SCH)]
                for j in range(JN):
                    r, sloc = j // SCH, (j % SCH) * P
                    h, off = sloc // VSL, sloc % VSL
                    v_t = vstr.tile([P, NQ], bf16, tag="vv",
                                    name=f"v_{eq}_{j}")
                    nc.scalar.dma_start(
                        out=v_t,
                        in_=v_all[h][r * VSL + off:r * VSL + off + P,
                                     eq * NQ:(eq + 1) * NQ])
                    for s in range(SCH):
                        nc.tensor.matmul(
                            o_ps[s], st_sb[:, j, s * P:(s + 1) * P], v_t,
                            start=(j == 0), stop=(j == JN - 1))
                for s in range(SCH):
                    # mul on Scalar, add on Vector: pipelined epilogue
                    o_stg = stg.tile([P, NQ], f32, tag="ostg",
                                     name=f"ostg_{eq}_{s}")
                    nc.scalar.activation(
                        o_stg, o_ps[s],
                        mybir.ActivationFunctionType.Identity,
                        scale=recip[:, s:s + 1])
                    nc.vector.tensor_tensor(
                        out=o_stg, in0=o_stg, in1=bv_sb,
                        op=mybir.AluOpType.add)
                    nc.sync.dma_start(
                        out=out[s * P:(s + 1) * P, eq * NQ:(eq + 1) * NQ],
                        in_=o_stg)

    nc.compile()
    return nc


def kernel(x, Wq, bq, Wk, bk, Wv, bv):
    from concourse.bass_utils import run_bass_kernel_spmd

    M = (np.asarray(Wq, dtype=np.float64)
         @ np.asarray(Wk, dtype=np.float64).T)           # [E, E] weight-only
    wc = SCALE * (np.asarray(Wk, dtype=np.float64)
                  @ np.asarray(bq, dtype=np.float64))    # [E] weight-only

    xb = x.astype(_BF16)                                  # [S, E]
    xt = np.ascontiguousarray(xb.T)                       # [E, S] bf16
    # x4d[j, p, c, s] = xT[c*128+p, j*128+s]: contiguous 512KB per j-tile
    x4d = np.ascontiguousarray(
        xt.reshape(ECH, P, JN, P).transpose(2, 1, 0, 3))

    def _pre(w):  # [e_in, e_out] -> [eo, p, c, n] so each eo-slice is contiguous
        return np.ascontiguousarray(
            w.astype(_BF16).reshape(ECH, P, ECH, P).transpose(2, 1, 0, 3))

    wmb = _pre(M.astype(np.float32))
    wvb = np.ascontiguousarray(Wv.astype(_BF16))
    wcb = np.ascontiguousarray(wc.astype(_BF16))
    bvf = np.ascontiguousarray(bv.astype(np.float32))

    in_maps = []
    for r in range(N_CORES):
        in_maps.append({
            "xt": np.ascontiguousarray(xt[:, r * SL:(r + 1) * SL]),
            "x4d": x4d,
            "wm": wmb, "wv": wvb, "wc": wcb, "bv": bvf,
        })

    nc = _build()
    res = run_bass_kernel_spmd(nc, in_maps, core_ids=list(range(N_CORES)))
    global LAST_RESULT
    LAST_RESULT = res
    return np.concatenate([res.results[r]["out"] for r in range(N_CORES)],
                          axis=0).astype(np.float32)


LAST_RESULT = None
